# revision 3
# baseline (speedup 1.0000x reference)
"""Masked-BCE (CenterNet-style) loss kernel for Trainium2, 8-core data parallel.

loss = sum(ppl * w) / (sum(w) * C)
  ppl = max(p,0) - p*t + log1p(exp(-|p|)) = softplus(p) - p*t
  w   = rand_mask | (max_c target > 0.5)          (per-pixel, broadcast over C)

Per-core restructure (exact for t in {0,1}):
  sum(ppl*w) = sum_x w(x)*S(x) - sum_{x,c} p*t      [since w*t == t]
  S(x) = sum_c softplus(p_c) = ln( prod_c (1 + e^{p_c}) )
  pos  = (sum_c t_c) > 0.5                          [since t binary]

Engine split per [128,2048] plane: ACT does Exp, DVE accumulates the (1+e)
product, POOL (gpsimd) does the p*t multiply, PE does all reductions: the
target channel-sum (identity matmul into PSUM) and the global sums of p*t,
w*S and w (ones-vector matmuls accumulating into single-bank PSUM rows).
Per-core partial sums are combined on the host (the final psum + divide).
"""

import numpy as np

import concourse.bacc as bacc
import concourse.mybir as mybir
from concourse import masks
from concourse.tile import TileContext
from concourse.bass_utils import run_bass_kernel_spmd

N, C, H, W = 32, 8, 512, 512
N_CORES = 8
NS = N // N_CORES          # samples per core
P = 128                    # SBUF partitions
F = (H * W) // P           # 2048 free elements per plane tile
NJ = F // 512              # 512-column matmul chunks per plane
FP32 = mybir.dt.float32
U8 = mybir.dt.uint8
Alu = mybir.AluOpType
Act = mybir.ActivationFunctionType


def _build(ns: int = NS):
    nc = bacc.Bacc("TRN2", target_bir_lowering=False, debug=False)
    pred = nc.dram_tensor("pred", [ns, C, H, W], FP32, kind="ExternalInput")
    target = nc.dram_tensor("target", [ns, C, H, W], FP32, kind="ExternalInput")
    rand = nc.dram_tensor("rand_mask", [ns, 1, H, W], U8, kind="ExternalInput")
    out = nc.dram_tensor("out", [1, 2], FP32, kind="ExternalOutput")

    predv = pred.ap().rearrange("n c (p a) w -> n c p (a w)", p=P)
    targv = target.ap().rearrange("n c (p a) w -> n c p (a w)", p=P)
    randv = rand.ap().rearrange("n c (p a) w -> n c p (a w)", p=P)

    with TileContext(nc) as tc:
        with (
            tc.tile_pool(name="io", bufs=3) as io,
            tc.tile_pool(name="work", bufs=2) as work,
            tc.tile_pool(name="scr", bufs=3) as scrp,
            tc.tile_pool(name="singles", bufs=1) as singles,
            tc.tile_pool(name="psum", bufs=1, space="PSUM") as psum,
        ):
            ident = singles.tile([P, P], FP32)
            masks.make_identity(nc, ident[:])
            ones = singles.tile([P, 1], FP32)
            nc.gpsimd.memset(ones[:], 1.0)

            # single-bank global accumulators (whole-kernel matmul groups)
            ps_pt = psum.tile([1, 512], FP32, tag="pt")    # sum p*t
            ps_ws = psum.tile([1, 512], FP32, tag="ws")    # sum w*S
            ps_cnt = psum.tile([1, 512], FP32, tag="cnt")  # sum w

            for n in range(ns):
                st = psum.tile([P, F], FP32, tag="st")     # channel-sum of t
                acc_p = work.tile([P, F], FP32, tag="accp")
                for c in range(C):
                    p_t = io.tile([P, F], FP32, tag="p")
                    nc.sync.dma_start(p_t[:], predv[n, c])
                    t_t = io.tile([P, F], FP32, tag="t")
                    nc.sync.dma_start(t_t[:], targv[n, c])

                    # ACT: e = exp(p)
                    e_t = io.tile([P, F], FP32, tag="e")
                    nc.scalar.activation(e_t[:], p_t[:], Act.Exp)

                    # DVE: acc_p = (e + 1) * acc_p   (prod of (1+e^p) over c)
                    if c == 0:
                        nc.vector.tensor_scalar_add(acc_p[:], e_t[:], 1.0)
                    else:
                        nc.vector.scalar_tensor_tensor(
                            out=acc_p[:], in0=e_t[:], scalar=1.0, in1=acc_p[:],
                            op0=Alu.add, op1=Alu.mult,
                        )

                    # POOL: pt = p * t
                    pt_t = scrp.tile([P, F], FP32, tag="pt")
                    nc.gpsimd.tensor_tensor(pt_t[:], p_t[:], t_t[:], Alu.mult)

                    first = (n == 0 and c == 0)
                    for j in range(NJ):
                        cols = slice(j * 512, (j + 1) * 512)
                        # PE: st += t (identity matmul, accumulate over c)
                        nc.tensor.matmul(st[:, cols], ident[:], t_t[:, cols],
                                         start=(c == 0), stop=(c == C - 1))
                        # PE: ps_pt += colsum(pt)
                        nc.tensor.matmul(ps_pt[:], ones[:], pt_t[:, cols],
                                         start=(first and j == 0), stop=False)

                # ---- per-sample epilogue ----
                rand_t = io.tile([P, F], U8, tag="rand")
                nc.sync.dma_start(rand_t[:], randv[n, 0])

                # S = ln(acc_p)
                s_t = work.tile([P, F], FP32, tag="s")
                nc.scalar.activation(s_t[:], acc_p[:], Act.Ln)

                # w = (st > 0.5) max rand    (st read from PSUM)
                w_t = work.tile([P, F], FP32, tag="w")
                nc.vector.scalar_tensor_tensor(
                    out=w_t[:], in0=st[:], scalar=0.5, in1=rand_t[:],
                    op0=Alu.is_gt, op1=Alu.max,
                )
                # ws = w * S
                ws_t = scrp.tile([P, F], FP32, tag="ws")
                nc.vector.tensor_tensor(ws_t[:], w_t[:], s_t[:], Alu.mult)

                last = (n == ns - 1)
                for j in range(NJ):
                    cols = slice(j * 512, (j + 1) * 512)
                    nc.tensor.matmul(ps_ws[:], ones[:], ws_t[:, cols],
                                     start=(n == 0 and j == 0),
                                     stop=(last and j == NJ - 1))
                    nc.tensor.matmul(ps_cnt[:], ones[:], w_t[:, cols],
                                     start=(n == 0 and j == 0),
                                     stop=(last and j == NJ - 1))

            # close ps_pt group: final accumulate with stop on a zero tile
            zero_t = singles.tile([P, 512], FP32)
            nc.gpsimd.memset(zero_t[:], 0.0)
            nc.tensor.matmul(ps_pt[:], ones[:], zero_t[:], start=False, stop=True)

            # ---- final extraction ----
            r_ws = singles.tile([1, 1], FP32)
            nc.vector.tensor_reduce(r_ws[:], ps_ws[:], axis=mybir.AxisListType.X,
                                    op=Alu.add)
            r_pt = singles.tile([1, 1], FP32)
            nc.vector.tensor_reduce(r_pt[:], ps_pt[:], axis=mybir.AxisListType.X,
                                    op=Alu.add)
            r_cnt = singles.tile([1, 1], FP32)
            nc.vector.tensor_reduce(r_cnt[:], ps_cnt[:], axis=mybir.AxisListType.X,
                                    op=Alu.add)
            out_sb = singles.tile([1, 2], FP32)
            nc.vector.tensor_tensor(out_sb[:, 0:1], r_ws[:], r_pt[:], Alu.subtract)
            nc.vector.tensor_copy(out_sb[:, 1:2], r_cnt[:])
            nc.sync.dma_start(out.ap(), out_sb[:])
    nc.compile()
    return nc


_NC_CACHE = {}


def _get_nc(ns: int = NS):
    if ns not in _NC_CACHE:
        _NC_CACHE[ns] = _build(ns)
    return _NC_CACHE[ns]


def kernel(pred, target, rand_mask):
    pred = np.asarray(pred, dtype=np.float32)
    target = np.asarray(target, dtype=np.float32)
    rand_mask = np.ascontiguousarray(np.asarray(rand_mask)).view(np.uint8)

    nc = _get_nc(NS)
    in_maps = []
    for i in range(N_CORES):
        sl = slice(i * NS, (i + 1) * NS)
        in_maps.append({
            "pred": np.ascontiguousarray(pred[sl]),
            "target": np.ascontiguousarray(target[sl]),
            "rand_mask": np.ascontiguousarray(rand_mask[sl]),
        })
    res = run_bass_kernel_spmd(nc, in_maps, list(range(N_CORES)))
    num = 0.0
    den = 0.0
    for r in res.results:
        o = r["out"].astype(np.float64)
        num += o[0, 0]
        den += o[0, 1]
    return np.float32(num / (den * C))
